# revision 6
# baseline (speedup 1.0000x reference)
"""Trainium2 Bass kernel for nn_DiscreteContinuousEncoder (DISCO S2 contraction).

Math (torch_harmonics _disco_s2_contraction, dense form):
    out[b,o,xo,po] = sum_{c,k,n} weight[o,c,k] * psi_vals[k,xo,n]
                     * x[b,c, row(k,xo,n), (col(k,xo,n) + 2*po) mod WIN]

Structure: row(k,xo,n) in [2*xo-2, 2*xo+2] (5-row band), col(k,xo,n) in
[0,16) (cc taps).  Host densifies psi+weight into per-latitude stencils
W2[xo, cc, c, dr, o] and pre-gathers x into a row-strip layout so the rhs of
every matmul is a pure strided view (step 2 = longitude downsampling).

Device: latitudes are processed 4 per block. SBUF tile x_t[(c,j), b*WP+w]
holds the block's 11 input rows j (= 2*xo0-2+j) per channel.  For each of 16
column taps cc, one PSUM-accumulating f32r matmul with K=(c,j)=88 and a
BANDED stationary operand lhsT[(c,j),(xq,o)] = W2[xo0+xq, cc, c, j-2*xq, o]
computes all 4 latitudes (M=128) for 360 longitudes; batch b is innermost so
4 consecutive matmuls share the stationary operand.

Sharding: Hout split 8 ways (48 rows/core incl. pad), batch looped on-core.
"""

from contextlib import ExitStack

import numpy as np

import bass_rust

B, CIN, COUT = 4, 8, 32
HIN, WIN = 721, 1440
HOUT, WOUT = 361, 720
KB = 9
NNZ = 32

NCORES = 8
XO_PER_CORE = 48           # 8*48 = 384 >= 361 (padded)
XQ = 4                     # latitudes per block
NBLK = XO_PER_CORE // XQ   # 12 blocks
NDR = 5                    # row band per latitude
NJ = 2 * XQ + 3            # 11 distinct input rows per block
KDIM = CIN * NJ            # 88, partition p = c*NJ + j
M = XQ * COUT              # 128, psum partition = xq*32+o
NCC = 16                   # column taps
NH = 2
NPO = WOUT // NH           # 360
WP = WIN + NCC             # 1456 (wrap columns appended)

_CACHE = {}


def _host_prep(x, psi_idx, psi_vals, weight):
    """Densify psi -> banded W2 stencils; pre-gather x into row strips."""
    x = np.ascontiguousarray(x, dtype=np.float32)
    psi_idx = np.asarray(psi_idx)
    psi_vals = np.asarray(psi_vals, dtype=np.float32)
    weight = np.asarray(weight, dtype=np.float32)

    rows = psi_idx // WIN
    cols = psi_idx % WIN
    dr = rows - (2 * np.arange(HOUT)[None, :, None] - 2)
    assert dr.min() >= 0 and dr.max() < NDR, (dr.min(), dr.max())
    assert cols.max() < NCC, cols.max()

    S = np.zeros((KB, HOUT, NDR, NCC), np.float32)
    k_i = np.repeat(np.arange(KB), HOUT * NNZ)
    xo_i = np.tile(np.repeat(np.arange(HOUT), NNZ), KB)
    np.add.at(S, (k_i, xo_i, dr.ravel(), cols.ravel()), psi_vals.ravel())

    # W2d[xo, cc, c, dr, o]
    W2d = np.einsum("ock,kxdm->xmcdo", weight, S, optimize=True)
    W2d_pad = np.zeros((NCORES * XO_PER_CORE, NCC, CIN, NDR, COUT), np.float32)
    W2d_pad[:HOUT] = W2d

    # w2[h]: [KDIM, NBLK*NCC*M]; banded: nonzero where j-2*xq in [0,NDR)
    w2 = np.zeros((NCORES, CIN, NJ, NBLK, NCC, XQ, COUT), np.float32)
    for xq in range(XQ):
        for d in range(NDR):
            j = 2 * xq + d
            xo = (
                np.arange(NCORES)[:, None] * XO_PER_CORE
                + np.arange(NBLK)[None, :] * XQ
                + xq
            )  # [h, blk]
            # W2d_pad[xo]: [h, blk, cc, c, dr=d, o] -> [h, c, blk, cc, o]
            w2[:, :, j, :, :, xq, :] = W2d_pad[xo][:, :, :, :, d, :].transpose(
                0, 3, 1, 2, 4
            )
    w2 = np.ascontiguousarray(w2.reshape(NCORES, KDIM, NBLK * NCC * M))

    # x padded: global rows -2..HIN+... -> index +2; columns wrapped to WP
    x_pad = np.zeros((B, CIN, 2 * NCORES * XO_PER_CORE + NJ, WP), np.float32)
    x_pad[:, :, 2 : 2 + HIN, :WIN] = x
    x_pad[:, :, 2 : 2 + HIN, WIN:] = x[:, :, :, : WP - WIN]

    # xs_dev[h]: [NBLK, KDIM, B*WP]; partition c*NJ+j holds padded row
    # 2*(48h+4blk)+j of channel c (global row 2*xo0-2+j), for each b.
    c_of = np.repeat(np.arange(CIN), NJ)   # [KDIM]
    j_of = np.tile(np.arange(NJ), CIN)
    xs = np.empty((NCORES, NBLK, KDIM, B * WP), np.float32)
    for h in range(NCORES):
        row_idx = (
            2 * (XO_PER_CORE * h + XQ * np.arange(NBLK)[:, None]) + j_of[None, :]
        )  # [NBLK, KDIM]
        gath = x_pad[:, c_of[None, :], row_idx, :]  # [B, NBLK, KDIM, WP]
        xs[h] = gath.transpose(1, 2, 0, 3).reshape(NBLK, KDIM, B * WP)
    return xs, w2


def _build(reps=1):
    import concourse.tile as tile
    from concourse import bacc, mybir

    nc = bacc.Bacc("TRN2", target_bir_lowering=False, debug=False,
                   num_devices=NCORES)
    f32r = mybir.dt.float32r
    f32 = mybir.dt.float32

    xs_ap = nc.dram_tensor("xs", [NBLK, KDIM, B * WP], f32r,
                           kind="ExternalInput").ap()
    w2_ap = nc.dram_tensor("w2", [KDIM, NBLK * NCC * M], f32r,
                           kind="ExternalInput").ap()
    out_ap = nc.dram_tensor("out", [B, COUT, XO_PER_CORE, WOUT], f32,
                            kind="ExternalOutput").ap()

    def body(ctx, tc):
        wpool = ctx.enter_context(tc.tile_pool(name="w2p", bufs=1))
        xpool = ctx.enter_context(tc.tile_pool(name="xp", bufs=2))
        spool = ctx.enter_context(tc.tile_pool(name="sp", bufs=8))
        pspool = ctx.enter_context(tc.tile_pool(name="psp", bufs=8, space="PSUM"))

        w2_sb = wpool.tile([KDIM, NBLK * NCC * M], f32r)
        nc.sync.dma_start(w2_sb[:], w2_ap[:])

        def compute(tc):
            for blk in range(NBLK):
                x_t = xpool.tile([KDIM, B * WP], f32r, tag="x_t")
                nc.sync.dma_start(x_t[:], xs_ap[blk])
                stages = [
                    spool.tile([M, WOUT], f32, tag="stage", name=f"stage_{blk}_{b}")
                    for b in range(B)
                ]
                for half in range(NH):
                    pss = [
                        pspool.tile([M, NPO], f32, tag="ps",
                                    name=f"ps_{blk}_{half}_{b}")
                        for b in range(B)
                    ]
                    for cc in range(NCC):
                        w_of = (blk * NCC + cc) * M
                        lhsT = w2_sb[:, w_of : w_of + M]
                        for b in range(B):
                            r_of = b * WP + cc + WOUT * half
                            rhs = x_t[:, r_of : r_of + 2 * NPO : 2]
                            nc.tensor.matmul(
                                pss[b][:, :], lhsT, rhs,
                                start=(cc == 0), stop=(cc == NCC - 1),
                            )
                    for b in range(B):
                        dst_sl = stages[b][:, half * NPO : (half + 1) * NPO]
                        if b % 2 == 0:
                            nc.vector.tensor_copy(dst_sl, pss[b][:, :])
                        else:
                            nc.scalar.copy(dst_sl, pss[b][:, :])
                for b in range(B):
                    dst = out_ap.copy()
                    dst.ap = bass_rust.VecI64Pair(
                        [[WOUT, XQ], [XO_PER_CORE * WOUT, COUT], [1, WOUT]]
                    )
                    dst.offset = (b * COUT * XO_PER_CORE + XQ * blk) * WOUT
                    nc.sync.dma_start(dst, stages[b][:])

        if reps == 1:
            compute(tc)
        else:
            with tc.For_i(0, reps, 1):
                compute(tc)

    with tile.TileContext(nc) as tc, ExitStack() as ctx:
        body(ctx, tc)
    nc.compile()
    return nc


def _get_nc(reps=1):
    if reps not in _CACHE:
        _CACHE[reps] = _build(reps)
    return _CACHE[reps]


def _run(xs, w2, reps=1):
    from concourse.bass_utils import run_bass_kernel_spmd

    nc = _get_nc(reps)
    in_maps = [{"xs": xs[h], "w2": w2[h]} for h in range(NCORES)]
    res = run_bass_kernel_spmd(
        nc, in_maps, core_ids=list(range(NCORES)), trace=False
    )
    outs = np.stack([res.results[h]["out"] for h in range(NCORES)])
    full = outs.transpose(1, 2, 0, 3, 4).reshape(
        B, COUT, NCORES * XO_PER_CORE, WOUT
    )
    return np.ascontiguousarray(full[:, :, :HOUT, :], dtype=np.float32)


def kernel(x, psi_idx, psi_vals, weight):
    xs, w2 = _host_prep(x, psi_idx, psi_vals, weight)
    return _run(xs, w2, reps=1)


# revision 7
# speedup vs baseline: 1.5842x; 1.5842x over previous
"""Trainium2 Bass kernel for nn_DiscreteContinuousEncoder (DISCO S2 contraction).

Math (torch_harmonics _disco_s2_contraction, dense form):
    out[b,o,xo,po] = sum_{c,k,n} weight[o,c,k] * psi_vals[k,xo,n]
                     * x[b,c, row(k,xo,n), (col(k,xo,n) + 2*po) mod WIN]

Structure: row(k,xo,n) in [2*xo-2, 2*xo+2] (5-row band), col(k,xo,n) in
[0,16) (cc taps).  Host densifies psi+weight into per-latitude stencils
W2[xo, cc, c, dr, o] and pre-gathers x into a row-strip layout so the rhs of
every matmul is a pure strided view (step 2 = longitude downsampling).

Device: latitudes are processed 4 per block. SBUF tile x_t[(c,j), b*WP+w]
holds the block's 11 input rows j (= 2*xo0-2+j) per channel.  For each of 16
column taps cc, one PSUM-accumulating f32r matmul with K=(c,j)=88 and a
BANDED stationary operand lhsT[(c,j),(xq,o)] = W2[xo0+xq, cc, c, j-2*xq, o]
computes all 4 latitudes (M=128) for 360 longitudes; batch b is innermost so
4 consecutive matmuls share the stationary operand.

Sharding: Hout split 8 ways (48 rows/core incl. pad), batch looped on-core.
"""

from contextlib import ExitStack

import numpy as np

import bass_rust

B, CIN, COUT = 4, 8, 32
HIN, WIN = 721, 1440
HOUT, WOUT = 361, 720
KB = 9
NNZ = 32

NCORES = 8
XO_PER_CORE = 48           # 8*48 = 384 >= 361 (padded)
XQ = 4                     # latitudes per block
NBLK = XO_PER_CORE // XQ   # 12 blocks
NDR = 5                    # row band per latitude
NJ = 2 * XQ + 3            # 11 distinct input rows per block
KDIM = CIN * NJ            # 88, partition p = c*NJ + j
M = XQ * COUT              # 128, psum partition = xq*32+o
NCC = 16                   # column taps
NH = 2
NPO = WOUT // NH           # 360
WP = WIN + NCC             # 1456 (wrap columns appended)

_CACHE = {}


def _host_prep(x, psi_idx, psi_vals, weight):
    """Densify psi -> banded W2 stencils; pre-gather x into row strips."""
    x = np.ascontiguousarray(x, dtype=np.float32)
    psi_idx = np.asarray(psi_idx)
    psi_vals = np.asarray(psi_vals, dtype=np.float32)
    weight = np.asarray(weight, dtype=np.float32)

    rows = psi_idx // WIN
    cols = psi_idx % WIN
    dr = rows - (2 * np.arange(HOUT)[None, :, None] - 2)
    assert dr.min() >= 0 and dr.max() < NDR, (dr.min(), dr.max())
    assert cols.max() < NCC, cols.max()

    S = np.zeros((KB, HOUT, NDR, NCC), np.float32)
    k_i = np.repeat(np.arange(KB), HOUT * NNZ)
    xo_i = np.tile(np.repeat(np.arange(HOUT), NNZ), KB)
    np.add.at(S, (k_i, xo_i, dr.ravel(), cols.ravel()), psi_vals.ravel())

    # W2d[xo, cc, c, dr, o]
    W2d = np.einsum("ock,kxdm->xmcdo", weight, S, optimize=True)
    W2d_pad = np.zeros((NCORES * XO_PER_CORE, NCC, CIN, NDR, COUT), np.float32)
    W2d_pad[:HOUT] = W2d

    # w2[h]: [KDIM, NBLK*NCC*M]; banded: nonzero where j-2*xq in [0,NDR)
    w2 = np.zeros((NCORES, CIN, NJ, NBLK, NCC, XQ, COUT), np.float32)
    for xq in range(XQ):
        for d in range(NDR):
            j = 2 * xq + d
            xo = (
                np.arange(NCORES)[:, None] * XO_PER_CORE
                + np.arange(NBLK)[None, :] * XQ
                + xq
            )  # [h, blk]
            # W2d_pad[xo]: [h, blk, cc, c, dr=d, o] -> [h, c, blk, cc, o]
            w2[:, :, j, :, :, xq, :] = W2d_pad[xo][:, :, :, :, d, :].transpose(
                0, 3, 1, 2, 4
            )
    w2 = np.ascontiguousarray(w2.reshape(NCORES, KDIM, NBLK * NCC * M))

    # x padded: global rows -2..HIN+... -> index +2; columns wrapped to WP
    x_pad = np.zeros((B, CIN, 2 * NCORES * XO_PER_CORE + NJ, WP), np.float32)
    x_pad[:, :, 2 : 2 + HIN, :WIN] = x
    x_pad[:, :, 2 : 2 + HIN, WIN:] = x[:, :, :, : WP - WIN]

    # xs_dev[h]: [NBLK, KDIM, B*WP]; partition c*NJ+j holds padded row
    # 2*(48h+4blk)+j of channel c (global row 2*xo0-2+j), for each b.
    c_of = np.repeat(np.arange(CIN), NJ)   # [KDIM]
    j_of = np.tile(np.arange(NJ), CIN)
    xs = np.empty((NCORES, NBLK, KDIM, B * WP), np.float32)
    for h in range(NCORES):
        row_idx = (
            2 * (XO_PER_CORE * h + XQ * np.arange(NBLK)[:, None]) + j_of[None, :]
        )  # [NBLK, KDIM]
        gath = x_pad[:, c_of[None, :], row_idx, :]  # [B, NBLK, KDIM, WP]
        xs[h] = gath.transpose(1, 2, 0, 3).reshape(NBLK, KDIM, B * WP)
    return xs, w2


def _build(reps=1):
    import concourse.tile as tile
    from concourse import bacc, mybir

    nc = bacc.Bacc("TRN2", target_bir_lowering=False, debug=False,
                   num_devices=NCORES)
    f32r = mybir.dt.float32r
    f32 = mybir.dt.float32

    xs_ap = nc.dram_tensor("xs", [NBLK, KDIM, B * WP], f32r,
                           kind="ExternalInput").ap()
    w2_ap = nc.dram_tensor("w2", [KDIM, NBLK * NCC * M], f32r,
                           kind="ExternalInput").ap()
    out_ap = nc.dram_tensor("out", [B, COUT, XO_PER_CORE, WOUT], f32,
                            kind="ExternalOutput").ap()

    def body(ctx, tc):
        wpool = ctx.enter_context(tc.tile_pool(name="w2p", bufs=1))
        xpool = ctx.enter_context(tc.tile_pool(name="xp", bufs=3))
        spool = ctx.enter_context(tc.tile_pool(name="sp", bufs=12))
        pspool = ctx.enter_context(tc.tile_pool(name="psp", bufs=8, space="PSUM"))

        w2_sb = wpool.tile([KDIM, NBLK * NCC * M], f32r)
        nc.sync.dma_start(w2_sb[:], w2_ap[:])

        def compute(tc):
            for blk in range(NBLK):
                x_t = xpool.tile([KDIM, B * WP], f32r, tag="x_t")
                nc.sync.dma_start(x_t[:], xs_ap[blk])
                stages = [
                    spool.tile([M, WOUT], f32, tag="stage", name=f"stage_{blk}_{b}")
                    for b in range(B)
                ]
                for half in range(NH):
                    pss = [
                        pspool.tile([M, NPO], f32, tag="ps",
                                    name=f"ps_{blk}_{half}_{b}")
                        for b in range(B)
                    ]
                    for cc in range(NCC):
                        w_of = (blk * NCC + cc) * M
                        lhsT = w2_sb[:, w_of : w_of + M]
                        for b in range(B):
                            r_of = b * WP + cc + WOUT * half
                            rhs = x_t[:, r_of : r_of + 2 * NPO : 2]
                            nc.tensor.matmul(
                                pss[b][:, :], lhsT, rhs,
                                start=(cc == 0), stop=(cc == NCC - 1),
                            )
                    for b in range(B):
                        dst_sl = stages[b][:, half * NPO : (half + 1) * NPO]
                        if b % 2 == 0:
                            nc.vector.tensor_copy(dst_sl, pss[b][:, :])
                        else:
                            nc.scalar.copy(dst_sl, pss[b][:, :])
                for b in range(B):
                    dst = out_ap.copy()
                    dst.ap = bass_rust.VecI64Pair(
                        [[WOUT, XQ], [XO_PER_CORE * WOUT, COUT], [1, WOUT]]
                    )
                    dst.offset = (b * COUT * XO_PER_CORE + XQ * blk) * WOUT
                    nc.sync.dma_start(dst, stages[b][:])

        if reps == 1:
            compute(tc)
        else:
            with tc.For_i(0, reps, 1):
                compute(tc)

    with tile.TileContext(nc) as tc, ExitStack() as ctx:
        body(ctx, tc)
    nc.compile()
    return nc


def _get_nc(reps=1):
    if reps not in _CACHE:
        _CACHE[reps] = _build(reps)
    return _CACHE[reps]


def _run(xs, w2, reps=1):
    from concourse.bass_utils import run_bass_kernel_spmd

    nc = _get_nc(reps)
    in_maps = [{"xs": xs[h], "w2": w2[h]} for h in range(NCORES)]
    res = run_bass_kernel_spmd(
        nc, in_maps, core_ids=list(range(NCORES)), trace=False
    )
    outs = np.stack([res.results[h]["out"] for h in range(NCORES)])
    full = outs.transpose(1, 2, 0, 3, 4).reshape(
        B, COUT, NCORES * XO_PER_CORE, WOUT
    )
    return np.ascontiguousarray(full[:, :, :HOUT, :], dtype=np.float32)


def kernel(x, psi_idx, psi_vals, weight):
    xs, w2 = _host_prep(x, psi_idx, psi_vals, weight)
    return _run(xs, w2, reps=1)
